# revision 25
# baseline (speedup 1.0000x reference)
"""Trainium2 Bass kernel for nn_LinearLoopLayer: out = x @ weight.T + bias.

x: (2048, 4096) f32, weight: (4096, 4096) f32, bias: (4096,) f32.
Sharding: 2 batch-halves x 4 out-feature-quarters across 8 NeuronCores.
Each core computes outT_shard[j, b] = sum_i wT[i, j] * xT[i, b] + bias[j]
with host-pre-transposed xT/wT so the contraction dim i is the SBUF
partition dim (no on-device transposes).

Default matmul dtype is bf16 (host-side cast): rel err ~1.3e-3 vs the
2e-2 gate, same PE rate as f32r (1 row/cycle on TRN2) but half the HBM
traffic. Flip with LINEAR_MM_DT=f32r / f32 (legacy build, streams wt).

bf16 build (v3) — measured findings this was built on:
- The MM stream (512 MMs x 512 rows, k-accumulated in PSUM) is purely
  rate-limited; weight-switch frequency, explicit ldweights, and MM
  granularity measurably don't matter. The only losses left are DMA
  stalls and pass-boundary PSUM drains.
- Each HWDGE ring (sync=SP, scalar=ACT) sustains ~160 GB/s and is
  FIFO per issuing engine, so load order must match consumption order
  and neither ring may be asked for more than ~150 GB/s in any window.
Structure: 4 passes over (j-half, batch-half), 4 PSUM banks per pass;
xt/wt host-packed into 256KB chunks ([128, CH*512]) that stream in
consumption order during the two js=0 passes (~120 GB/s per ring) and
stay SBUF-resident (64KB + 64KB per partition) for the js=1 passes;
bias-adds (DVE) + out stores of each pass drain under the next pass.
LINEAR_OUT_RING=sync|scalar picks the out-store ring (A/B'd).
"""

import os
import sys

import numpy as np

sys.path.insert(0, "/opt/trn_rl_repo")

import concourse.mybir as mybir
from concourse import bacc, tile
from concourse.bass_utils import run_bass_kernel_spmd

P = 128
B, K, J = 2048, 4096, 4096
NCORES = 8
B_SPLIT, J_SPLIT = 2, 4
BL, JL = B // B_SPLIT, J // J_SPLIT  # per-core local batch / out-features
KT = K // P  # contraction tiles
MV = 512  # moving-dim block: psum output capped at one 2KB bank (512 f32)
NB = BL // MV
JS = JL // 512  # j-half blocks (512 features) per core
JSUB = 512 // P  # 128-feature psum row-blocks per j-half
CH = 2  # k-tiles per packed DMA chunk
NXC = KT // CH  # xt chunks per batch-half (16): [128, CH*512] bf16 = 256KB
NWC = KT // CH  # wt chunks per j-half (16): [128, CH*512] bf16 = 256KB

_DT_BY_NAME = {
    "f32": mybir.dt.float32,
    "f32r": mybir.dt.float32r,
    "bf16": mybir.dt.bfloat16,
}
_MM_DT_NAME = os.environ.get("LINEAR_MM_DT", "bf16")


def _rep_ctx(tc, nrep):
    from contextlib import nullcontext

    return (
        tc.For_i(0, nrep, 1, hint_engines=(mybir.EngineType.PE,))
        if nrep > 1
        else nullcontext()
    )


def _build_bf16(nrep=1, out_ring=None):
    """v3 build (bf16 only): 4 passes of (j-half, batch-half), 4 PSUM
    banks per pass. Each pass consumes 4MB xt + 4MB wt spread over its
    ~34us (~120 GB/s per ring, under the ~160 GB/s per-ring rate), so
    input DMA never paces the PE; bias adds drain under the next pass.
    xt/wt are host-packed [32, 128, CH*512] (256KB chunks) and stay
    SBUF-resident (64KB + 64KB per partition)."""
    mm_dt = mybir.dt.bfloat16
    out_ring = out_ring or os.environ.get("LINEAR_OUT_RING", "sync")
    nc = bacc.Bacc(None, target_bir_lowering=False)
    # xt chunk (bb, c): k-tiles CH*c..CH*c+CH-1, batch cols bb*512+:512
    xtp = nc.declare_dram_parameter(
        "xtp", [NB * NXC, P, CH * MV], mm_dt, isOutput=False
    )
    # wt chunk (js, c): k-tiles CH*c..., feature cols js*512+:512
    wtp = nc.declare_dram_parameter(
        "wtp", [JS * NWC, P, CH * 512], mm_dt, isOutput=False
    )
    biasT = nc.declare_dram_parameter(
        "biasT", [P, JL // P], mybir.dt.float32, isOutput=False
    )
    # out in bf16: halves store-side SBUF/HBM traffic (host upcasts after
    # gather); adds ~2e-3 output rounding against the 2e-2 gate
    out = nc.declare_dram_parameter("out", [JL, BL], mm_dt, isOutput=True)

    f32 = mybir.dt.float32
    with tile.TileContext(nc) as tc:
        with (
            tc.tile_pool(name="xtp", bufs=NB * NXC) as xt_pool,
            tc.tile_pool(name="wtp", bufs=JS * NWC) as wt_pool,
            tc.tile_pool(name="outp", bufs=8) as out_pool,
            tc.tile_pool(name="biasp", bufs=1) as bias_pool,
            tc.tile_pool(name="psum", bufs=8, space="PSUM") as psum_pool,
        ):
            with _rep_ctx(tc, nrep):
                bias_sb = bias_pool.tile([P, JL // P], f32)
                nc.scalar.dma_start(bias_sb[:], biasT[:, :])

                xt_ch = [[None] * NXC for _ in range(NB)]
                wt_ch = [[None] * NWC for _ in range(JS)]

                def xt_rhs(bb, i):
                    c, s = divmod(i, CH)
                    return xt_ch[bb][c][:, s * MV : (s + 1) * MV]

                def wt_lhs(js, i, jsub):
                    c, s = divmod(i, CH)
                    return wt_ch[js][c][:, s * 512 + jsub * P : s * 512 + (jsub + 1) * P]

                for js in range(JS):
                    for bb in range(NB):
                        ps = [
                            psum_pool.tile([P, MV], f32, name="ps")
                            for _ in range(JSUB)
                        ]
                        for i in range(KT):
                            if js == 0 and i % CH == 0:
                                c = i // CH
                                # this pass's batch-half of xt on the sync ring;
                                # first chunk split in half so the first MM
                                # only waits on 128KB (subtile deps)
                                t = xt_pool.tile([P, CH * MV], mm_dt, name="xt")
                                xsrc = xtp[bb * NXC + c]
                                if bb == 0 and c == 0:
                                    nc.sync.dma_start(t[:, :MV], xsrc[:, :MV])
                                    nc.sync.dma_start(t[:, MV:], xsrc[:, MV:])
                                else:
                                    nc.sync.dma_start(t[:], xsrc)
                                xt_ch[bb][c] = t
                                # wt j-half 0 rides pass (0,0); j-half 1
                                # prefetches in pass (0,1)'s shadow (NB == JS)
                                jw = bb
                                tw = wt_pool.tile([P, CH * 512], mm_dt, name="wt")
                                wsrc = wtp[jw * NWC + c]
                                if bb == 0 and c == 0:
                                    nc.scalar.dma_start(tw[:, :512], wsrc[:, :512])
                                    nc.scalar.dma_start(tw[:, 512:], wsrc[:, 512:])
                                else:
                                    nc.scalar.dma_start(tw[:], wsrc)
                                wt_ch[jw][c] = tw
                            for jsub in range(JSUB):
                                nc.tensor.matmul(
                                    ps[jsub][:],
                                    wt_lhs(js, i, jsub),
                                    xt_rhs(bb, i),
                                    start=(i == 0),
                                    stop=(i == KT - 1),
                                )
                        for jsub in range(JSUB):
                            jb = js * JSUB + jsub
                            o = out_pool.tile([P, MV], mm_dt, name="o")
                            nc.vector.tensor_scalar_add(
                                o[:], ps[jsub][:], bias_sb[:, jb : jb + 1]
                            )
                            store_eng = nc.sync if out_ring == "sync" else nc.scalar
                            store_eng.dma_start(
                                out[jb * P : (jb + 1) * P, bb * MV : (bb + 1) * MV],
                                o[:],
                            )
    nc.finalize()
    return nc


def _build_f32(mm_dt, nrep=1):
    """Legacy build for f32/f32r (wt streamed, not resident)."""
    nc = bacc.Bacc(None, target_bir_lowering=False)
    xt = nc.declare_dram_parameter("xt", [K, BL], mm_dt, isOutput=False)
    wt = nc.declare_dram_parameter("wt", [K, JL], mm_dt, isOutput=False)
    biasT = nc.declare_dram_parameter(
        "biasT", [P, JL // P], mybir.dt.float32, isOutput=False
    )
    out = nc.declare_dram_parameter("out", [JL, BL], mybir.dt.float32, isOutput=True)

    f32 = mybir.dt.float32
    with tile.TileContext(nc) as tc:
        with (
            tc.tile_pool(name="xtp", bufs=KT) as xt_pool,
            tc.tile_pool(name="wtp", bufs=6) as wt_pool,
            tc.tile_pool(name="outp", bufs=4) as out_pool,
            tc.tile_pool(name="biasp", bufs=1) as bias_pool,
            tc.tile_pool(name="psum", bufs=8, space="PSUM") as psum_pool,
        ):
            with _rep_ctx(tc, nrep):
                bias_sb = bias_pool.tile([P, JL // P], f32)
                nc.sync.dma_start(bias_sb[:], biasT[:, :])

                xt_tiles = [None] * KT
                for js in range(JS):
                    ps = [
                        [psum_pool.tile([P, MV], f32, name="ps") for bb in range(NB)]
                        for jsub in range(JSUB)
                    ]
                    for i in range(KT):
                        if js == 0:
                            t = xt_pool.tile([P, BL], mm_dt, name="xt")
                            nc.sync.dma_start(t[:], xt[i * P : (i + 1) * P, :])
                            xt_tiles[i] = t
                        wt_t = wt_pool.tile([P, 512], mm_dt, name="wt")
                        nc.scalar.dma_start(
                            wt_t[:], wt[i * P : (i + 1) * P, js * 512 : (js + 1) * 512]
                        )
                        for jsub in range(JSUB):
                            for bb in range(NB):
                                nc.tensor.matmul(
                                    ps[jsub][bb][:],
                                    wt_t[:, jsub * P : (jsub + 1) * P],
                                    xt_tiles[i][:, bb * MV : (bb + 1) * MV],
                                    start=(i == 0),
                                    stop=(i == KT - 1),
                                )
                    for jsub in range(JSUB):
                        jb = js * JSUB + jsub
                        for bb in range(NB):
                            o = out_pool.tile([P, MV], f32, name="o")
                            nc.vector.tensor_scalar_add(
                                o[:], ps[jsub][bb][:], bias_sb[:, jb : jb + 1]
                            )
                            nc.sync.dma_start(
                                out[jb * P : (jb + 1) * P, bb * MV : (bb + 1) * MV],
                                o[:],
                            )
    nc.finalize()
    return nc


def _build(mm_dt, nrep=1):
    if mm_dt == mybir.dt.bfloat16:
        return _build_bf16(nrep=nrep)
    return _build_f32(mm_dt, nrep=nrep)


_NC_CACHE = {}


def _get_nc(mm_dt_name, nrep=1):
    key = (mm_dt_name, nrep)
    if key not in _NC_CACHE:
        _NC_CACHE[key] = _build(_DT_BY_NAME[mm_dt_name], nrep=nrep)
    return _NC_CACHE[key]


def _pack_xt(xT_shard):
    """[K, BL] -> [NB*NXC, 128, CH*MV], batch-half-major: chunk (bb, c),
    partition p, sub s, col m holds xT[(c*CH+s)*128 + p, bb*MV + m]."""
    a = xT_shard.reshape(NXC, CH, P, NB, MV)
    a = a.transpose(3, 0, 2, 1, 4)  # [bb, c, p, s, m]
    return np.ascontiguousarray(a.reshape(NB * NXC, P, CH * MV))


def _pack_wt(wT_shard):
    """[K, JL] -> [JS*NWC, 128, CH*512], j-half-major."""
    a = wT_shard.reshape(NWC, CH, P, JS, 512)  # [c, s, p, js, j]
    a = a.transpose(3, 0, 2, 1, 4)  # [js, c, p, s, j]
    return np.ascontiguousarray(a.reshape(JS * NWC, P, CH * 512))


def _make_in_maps(x, weight, bias, mm_dt):
    np_dt = mybir.dt.np(mm_dt)
    x = np.asarray(x, dtype=np.float32)
    if x.ndim == 4:
        x = x.reshape(x.shape[0], -1)
    weight = np.asarray(weight, dtype=np.float32)
    bias = np.asarray(bias, dtype=np.float32)
    xT = np.ascontiguousarray(x.T.astype(np_dt))  # [K, B]
    wT = np.ascontiguousarray(weight.T.astype(np_dt))  # [K, J]
    packed = mm_dt == mybir.dt.bfloat16
    in_maps = []
    for c in range(NCORES):
        bh, jq = divmod(c, J_SPLIT)
        bq = bias[jq * JL : (jq + 1) * JL]
        xs = xT[:, bh * BL : (bh + 1) * BL]
        ws = wT[:, jq * JL : (jq + 1) * JL]
        m = {"biasT": np.ascontiguousarray(bq.reshape(JL // P, P).T)}
        if packed:
            m["xtp"] = _pack_xt(xs)
            m["wtp"] = _pack_wt(ws)
        else:
            m["xt"] = np.ascontiguousarray(xs)
            m["wt"] = np.ascontiguousarray(ws)
        in_maps.append(m)
    return in_maps


def _assemble(results):
    out = np.empty((B, J), dtype=np.float32)
    for c in range(NCORES):
        bh, jq = divmod(c, J_SPLIT)
        out[bh * BL : (bh + 1) * BL, jq * JL : (jq + 1) * JL] = results[c]["out"].T
    return out


def run(x, weight, bias, mm_dt_name=None, trace=False, nrep=1, **kwargs):
    mm_dt_name = mm_dt_name or _MM_DT_NAME
    nc = _get_nc(mm_dt_name, nrep=nrep)
    in_maps = _make_in_maps(x, weight, bias, _DT_BY_NAME[mm_dt_name])
    res = run_bass_kernel_spmd(
        nc, in_maps, core_ids=list(range(NCORES)), trace=trace, **kwargs
    )
    return _assemble(res.results), res


def kernel(x, weight, bias):
    out, _ = run(x, weight, bias)
    return out
